# revision 13
# baseline (speedup 1.0000x reference)
"""GroupedQueryAttention kernel for 8 Trainium2 NeuronCores.

Sharding: tensor-parallel over KV groups (core c owns group c = 4 query
heads x 64): column shards of w_q/w_k/w_v, row shard of w_o; x
replicated (bf16, pre-transposed, partition-major); each core writes a
partial bf16 output that the host sums.

v2 design (vs the 290us flat-pipeline baseline):
- The ACT (Scalar) engine's softmax exp stream is the long pole:
  128 ACTIVATEs x ~1.15us = ~147us that cannot be reduced (exp is
  ACT-only, 1 elem/cycle/lane).  So the kernel starts that stream as
  early as the DMA allows (~19us instead of ~55us) and hides ALL other
  PE work (KV/Q projections for chunks 1-3, Q(qt1), V transposes,
  o-proj) inside the exp-paced slack via a static DMA-aware filler
  schedule.
- x is DMA'd seq-chunk-major so the chunk-0 KV+Q0 projections stream
  against the DMA and the first scores fire as soon as chunk 0 lands.
- A dummy exp ACTIVATE at t=0 pulls the ~2.7us ACT table load into the
  DMA-startup shadow.
- Steady state: ACT does nothing but exp; all psum evacuations (y
  tiles, kT/vT/qT casts, epilogue) run on the DVE; tail o-proj casts
  alternate DVE/ACT.
- PSUM budget (8 banks): scores 2x[128,2,512] (4) + AV 2x[65,512] (2)
  + filler/o-proj mip ring x2 (2).  The mip ring is strict
  round-robin, so filler emission follows a parity-safe order: held
  projection accumulators alternate slots and transients come in
  windows where the previous occupant is free.

Layouts per core (S=2048, D=2048, 4 heads of 64):
  xT_sb  [128, 16, 2048] bf16   x^T k-tiles (host partition-major)
  qT_sb  [128, 2, 2048]  bf16   Q^T; head h -> partitions 64*(h%2), slot h//2
  kT_sb  [128, 2048]     bf16   K^T duplicated on both partition halves
  v1_sb  [128, 16, 65]   bf16   [V | ones] natural layout per sk tile
  oT_sb  [128, 2, 2048]  bf16   normalized attention out (same map as qT)
  out    [2048, 2048]    bf16   partial output, host-summed
"""

import numpy as np

S = 2048
D = 2048
N_CORES = 8
HD = 64
HPG = 4
QDIM = HPG * HD           # 256
SCALE = 1.0 / 8.0         # 1/sqrt(HD)
SQC = 512                 # seq chunk (psum bank width in f32)
NCH = S // SQC            # 4
T = S // 128              # 16 sk tiles
KO = D // 128             # 16 contraction tiles
QT = QDIM // 128          # 2 q partition tiles (= head pairs)

_compiled = {}


def _noldw(bi):
    bi.ins.ldweights = False
    return bi


def build_gqa(debug=False):
    import concourse.tile as tile
    from concourse import bacc, mybir
    from concourse.masks import make_identity
    from contextlib import ExitStack

    f32 = mybir.dt.float32
    bf16 = mybir.dt.bfloat16
    EXP = mybir.ActivationFunctionType.Exp

    nc = bacc.Bacc(None, target_bir_lowering=False, debug=debug)
    xTp = nc.declare_dram_parameter("xTp", [128, KO * S], bf16, isOutput=False)
    wqp0 = nc.declare_dram_parameter("wqp0", [128, KO * 128], bf16, isOutput=False)
    wqp1 = nc.declare_dram_parameter("wqp1", [128, KO * 128], bf16, isOutput=False)
    wkvp = nc.declare_dram_parameter("wkvp", [128, KO * 2 * HD], bf16, isOutput=False)
    wop = nc.declare_dram_parameter("wop", [128, QT * D], bf16, isOutput=False)
    out = nc.declare_dram_parameter("out", [S, D], bf16, isOutput=True)
    # qt0-only partial of the last chunk's o-proj (host adds it to out's
    # qt1-only last-chunk rows) — lets item7 hoist half the o-proj tail
    out2 = nc.declare_dram_parameter("out2", [SQC, D], bf16, isOutput=True)

    with tile.TileContext(nc) as tc, ExitStack() as ctx:
        const = ctx.enter_context(tc.tile_pool(name="const", bufs=1))
        persist = ctx.enter_context(tc.tile_pool(name="persist", bufs=1))

        ident = const.tile([128, 128], bf16)
        ones_bf = const.tile([1, HD], bf16)
        bias_exp = const.tile([128, 1], f32)
        warm = const.tile([128, 1], f32)
        # dummy exp ASAP: forces the ACT table load at t~0, under the DMA
        # startup shadow (otherwise it serializes before the first real exp)
        nc.vector.memset(bias_exp, -8.0)
        nc.scalar.activation(out=warm, in_=bias_exp, func=EXP, bias=0.0, scale=1.0)
        make_identity(nc, ident)
        nc.vector.memset(ones_bf, 1.0)

        xT_sb = persist.tile([128, KO, S], bf16)
        qT_sb = persist.tile([128, QT, S], bf16)
        kT_sb = persist.tile([128, S], bf16)
        v1_sb = persist.tile([128, T, HD + 1], bf16)
        oT_sb = persist.tile([128, QT, S], bf16)
        wkv_sb = persist.tile([128, KO, 2 * HD], bf16)
        wq_sb = persist.tile([128, QT, KO, 128], bf16)
        wo_sb = persist.tile([128, QT, D], bf16)
        vT_tmp = persist.tile([64, S], bf16)

        nc.vector.memset(v1_sb[:, :, HD:HD + 1], 1.0)

        # ---------------- input DMAs ------------------------------------
        # seq-chunk-major so compute can stream against DMA arrival.
        # Few, large descriptors (sync-engine issue is ~0.6us each).
        xTr = xTp[:].rearrange("p (ko s) -> p ko s", ko=KO)

        def dma_x(ch, ko_lo, ko_hi):
            cs = slice(ch * SQC, (ch + 1) * SQC)
            nc.sync.dma_start(
                out=xT_sb[:, ko_lo:ko_hi, cs], in_=xTr[:, ko_lo:ko_hi, cs])

        # chunk-0 weights+x interleaved in 4-ko groups so the first kv/q0
        # matmuls start as soon as ~0.8MB has landed
        wkvr = wkvp[:].rearrange("p (ko m) -> p ko m", ko=KO)
        wq0r = wqp0[:].rearrange("p (ko m) -> p ko m", ko=KO)
        for g in range(4):
            nc.sync.dma_start(out=wkv_sb[:, 4 * g:4 * g + 4], in_=wkvr[:, 4 * g:4 * g + 4])
            nc.sync.dma_start(out=wq_sb[:, 0, 4 * g:4 * g + 4], in_=wq0r[:, 4 * g:4 * g + 4])
            dma_x(0, 4 * g, 4 * g + 4)
        dma_x(1, 0, 8)
        dma_x(1, 8, KO)
        nc.sync.dma_start(
            out=wq_sb[:, 1], in_=wqp1[:].rearrange("p (ko m) -> p ko m", ko=KO))
        dma_x(2, 0, 8)
        dma_x(2, 8, KO)
        dma_x(3, 0, 8)
        dma_x(3, 8, KO)
        nc.sync.dma_start(out=wo_sb, in_=wop[:].rearrange("p (qt m) -> p qt m", qt=QT))

        # ---------------- pools -----------------------------------------
        scps = ctx.enter_context(tc.tile_pool(name="scps", bufs=2, space="PSUM"))
        avps = ctx.enter_context(tc.tile_pool(name="avps", bufs=2, space="PSUM"))
        mips = ctx.enter_context(tc.tile_pool(name="mips", bufs=2, space="PSUM"))
        eps = ctx.enter_context(tc.tile_pool(name="eps", bufs=8))
        p2ev = ctx.enter_context(tc.tile_pool(name="p2ev", bufs=4))
        ypool = ctx.enter_context(tc.tile_pool(name="ypool", bufs=4))

        # ---------------- filler building blocks ------------------------
        proj_state = {}

        def kv_slice(ch, kos):
            cs = slice(ch * SQC, (ch + 1) * SQC)
            if kos[0] == 0:
                proj_state[("kv", ch)] = mips.tile(
                    [128, SQC], f32, name=f"kv{ch}", tag="mip")
            ps = proj_state[("kv", ch)]
            for ko in kos:
                nc.tensor.matmul(
                    ps, wkv_sb[:, ko, :], xT_sb[:, ko, cs],
                    start=(ko == 0), stop=(ko == KO - 1))

        def kv_cast(ch):
            cs = slice(ch * SQC, (ch + 1) * SQC)
            ps = proj_state.pop(("kv", ch))
            nc.vector.tensor_copy(out=kT_sb[0:64, cs], in_=ps[0:64, :])
            nc.vector.tensor_copy(out=kT_sb[64:128, cs], in_=ps[0:64, :])
            nc.vector.tensor_copy(out=vT_tmp[:, cs], in_=ps[64:128, :])

        def q_slice(qt, ch, kos):
            cs = slice(ch * SQC, (ch + 1) * SQC)
            if kos[0] == 0:
                proj_state[("q", qt, ch)] = mips.tile(
                    [128, SQC], f32, name=f"q{qt}{ch}", tag="mip")
            ps = proj_state[("q", qt, ch)]
            for ko in kos:
                nc.tensor.matmul(
                    ps, wq_sb[:, qt, ko, :], xT_sb[:, ko, cs],
                    start=(ko == 0), stop=(ko == KO - 1))

        def q_cast(qt, ch):
            cs = slice(ch * SQC, (ch + 1) * SQC)
            ps = proj_state.pop(("q", qt, ch))
            nc.vector.tensor_copy(out=qT_sb[:, qt, cs], in_=ps)

        def vtrans(j):
            pt = mips.tile([128, HD], bf16, name="pt", tag="mip")
            nc.tensor.transpose(
                pt, vT_tmp[:, j * 128:(j + 1) * 128], ident[0:64, 0:64])
            nc.vector.tensor_copy(out=v1_sb[:, j, 0:HD], in_=pt)

        def oproj_half(t, oc, qt, tail_idx=-1):
            """Single-qt partial o-proj for the last chunk: qt0 goes to
            out2 (host adds), qt1 to the normal out rows."""
            ns = slice(oc * SQC, (oc + 1) * SQC)
            if tail_idx >= 0:
                pool, tag = [(mips, "mip"), (avps, "av"), (scps, "sc")][tail_idx % 3]
                py = pool.tile([128, SQC], f32, name="pyh", tag=tag)
            else:
                py = mips.tile([128, SQC], f32, name="pyh", tag="mip")
            nc.tensor.matmul(
                py, oT_sb[:, qt, t * 128:(t + 1) * 128], wo_sb[:, qt, ns],
                start=True, stop=True)
            y_sb = ypool.tile([128, SQC], bf16, name="y_sb")
            with nc.allow_low_precision(reason="bf16 partial output"):
                if tail_idx >= 0 and tail_idx % 2 == 1:
                    nc.scalar.copy(out=y_sb, in_=py)
                else:
                    nc.vector.tensor_copy(out=y_sb, in_=py)
            if qt == 0:
                dst = out2[:].rearrange("(t p) n -> p t n", p=128)[:, t - 12, ns]
            else:
                dst = out[:].rearrange("(t p) n -> p t n", p=128)[:, t, ns]
            nc.sync.dma_start(out=dst, in_=y_sb)

        def oproj(t, oc, tail_idx=-1):
            ns = slice(oc * SQC, (oc + 1) * SQC)
            if tail_idx >= 0:
                pool, tag = [(mips, "mip"), (avps, "av"), (scps, "sc")][tail_idx % 3]
                py = pool.tile([128, SQC], f32, name="py", tag=tag)
            else:
                py = mips.tile([128, SQC], f32, name="py", tag="mip")
            for qt in range(QT):
                nc.tensor.matmul(
                    py, oT_sb[:, qt, t * 128:(t + 1) * 128], wo_sb[:, qt, ns],
                    start=(qt == 0), stop=(qt == QT - 1))
            y_sb = ypool.tile([128, SQC], bf16, name="y_sb")
            with nc.allow_low_precision(reason="bf16 partial output"):
                if tail_idx >= 0 and tail_idx % 2 == 1:
                    nc.scalar.copy(out=y_sb, in_=py)
                else:
                    nc.vector.tensor_copy(out=y_sb, in_=py)
            nc.sync.dma_start(
                out=out[:].rearrange("(t p) n -> p t n", p=128)[:, t, ns],
                in_=y_sb)

        # ---------------- static filler schedule ------------------------
        # pre[(item, sk)] runs BEFORE the slot's score-lookahead emit (so
        # kv casts land ahead of the scores that read them — the PE FIFO
        # is in-order, a later-emitted dependency would deadlock it).
        # post[(item, sk)] runs after the slot's AV matmuls.
        pre, post = {}, {}

        def addp(d, item, sk, fn):
            d.setdefault((item, sk), []).append(fn)

        # kv chunks 1-3: front-loaded, DMA-gated (x chunk c lands ~17+5.4c us)
        addp(pre, 0, 0, lambda: kv_slice(1, range(0, 8)))
        addp(pre, 0, 1, lambda: kv_slice(1, range(8, KO)))
        addp(pre, 0, 1, lambda: kv_cast(1))
        addp(pre, 0, 4, lambda: kv_slice(2, range(0, 8)))
        addp(pre, 0, 5, lambda: kv_slice(2, range(8, KO)))
        addp(pre, 0, 6, lambda: kv_cast(2))
        addp(pre, 0, 8, lambda: kv_slice(3, range(0, 8)))
        addp(pre, 0, 9, lambda: kv_slice(3, range(8, KO)))
        addp(pre, 0, 10, lambda: kv_cast(3))
        # V transposes: v1[j] must exist before AV at (0, j)
        for j in range(4, 8):
            addp(post, 0, 1, lambda j=j: vtrans(j))
        for j in range(8, 12):
            addp(post, 0, 6, lambda j=j: vtrans(j))
        for j in range(12, 16):
            addp(post, 0, 10, lambda j=j: vtrans(j))
        # Q projections for later items (parity-safe slots, see docstring)
        addp(post, 0, 11, lambda: q_slice(1, 0, range(0, 6)))
        addp(post, 0, 12, lambda: q_slice(1, 0, range(6, 12)))
        addp(post, 0, 13, lambda: q_slice(1, 0, range(12, KO)))
        addp(post, 0, 13, lambda: q_cast(1, 0))
        addp(post, 0, 12, lambda: q_slice(0, 1, range(0, 6)))
        addp(post, 0, 13, lambda: q_slice(0, 1, range(6, 12)))
        addp(post, 0, 14, lambda: q_slice(0, 1, range(12, KO)))
        addp(post, 0, 14, lambda: q_cast(0, 1))
        addp(post, 1, 1, lambda: q_slice(1, 1, range(0, 6)))
        addp(post, 1, 2, lambda: q_slice(1, 1, range(6, 12)))
        addp(post, 1, 3, lambda: q_slice(1, 1, range(12, KO)))
        addp(post, 1, 3, lambda: q_cast(1, 1))
        addp(post, 1, 6, lambda: q_slice(0, 2, range(0, 6)))
        addp(post, 1, 7, lambda: q_slice(0, 2, range(6, 12)))
        addp(post, 1, 8, lambda: q_slice(0, 2, range(12, KO)))
        addp(post, 1, 8, lambda: q_cast(0, 2))
        addp(post, 2, 1, lambda: q_slice(1, 2, range(0, 6)))
        addp(post, 2, 2, lambda: q_slice(1, 2, range(6, 12)))
        addp(post, 2, 3, lambda: q_slice(1, 2, range(12, KO)))
        addp(post, 2, 3, lambda: q_cast(1, 2))
        addp(post, 3, 1, lambda: q_slice(0, 3, range(0, 6)))
        addp(post, 3, 2, lambda: q_slice(0, 3, range(6, 12)))
        addp(post, 3, 3, lambda: q_slice(0, 3, range(12, KO)))
        addp(post, 3, 3, lambda: q_cast(0, 3))
        addp(post, 4, 1, lambda: q_slice(1, 3, range(0, 6)))
        addp(post, 4, 2, lambda: q_slice(1, 3, range(6, 12)))
        addp(post, 4, 3, lambda: q_slice(1, 3, range(12, KO)))
        addp(post, 4, 3, lambda: q_cast(1, 3))
        # o-proj: ch0 spread thin over items 2-4 (their q-slice slots are
        # light), ch1 -> item5, ch2 -> item6 (one per sk); ch3's qt0-only
        # halves run during item7 (oT-ch3-qt0 exists after item6's
        # epilogue at (7,1)), qt1 halves go to the tail.
        t0 = [(tt, oc) for tt in range(4) for oc in range(NCH)]
        slots0 = [(2, sk) for sk in (5, 7, 9, 11, 13, 15)] + \
                 [(3, sk) for sk in (5, 7, 9, 11, 13, 15)] + \
                 [(4, sk) for sk in (5, 7, 9, 11)]
        for (tt, oc), (it, sk) in zip(t0, slots0):
            addp(post, it, sk, lambda tt=tt, oc=oc: oproj(tt, oc))
        for i, (t, oc) in enumerate([(4 + tt, oc) for tt in range(4) for oc in range(NCH)]):
            addp(post, 5, i, lambda t=t, oc=oc: oproj(t, oc))
        # ch2: first slot must follow epi(item5) at (6,1) -> start at sk2
        t2 = [(8 + tt, oc) for tt in range(4) for oc in range(NCH)]
        for i, (t, oc) in enumerate(t2[:14]):
            addp(post, 6, i + 2, lambda t=t, oc=oc: oproj(t, oc))
        for i, (t, oc) in enumerate(t2[14:]):
            addp(post, 7, i, lambda t=t, oc=oc: oproj(t, oc))
        th = [(12 + tt, oc) for tt in range(4) for oc in range(NCH)]
        for i, (t, oc) in enumerate(th[:14]):
            addp(post, 7, i + 2, lambda t=t, oc=oc: oproj_half(t, oc, 0))

        # ---------------- prologue: chunk-0 KV + Q0, DMA-paced ----------
        for g in range(4):
            kv_slice(0, range(4 * g, 4 * g + 4))
            q_slice(0, 0, range(4 * g, 4 * g + 4))
        kv_cast(0)
        q_cast(0, 0)
        for j in range(4):
            vtrans(j)

        # ---------------- main loop: one flat pipeline -------------------
        items = [(ch, qt, sk)
                 for ch in range(NCH) for qt in range(QT) for sk in range(T)]
        sc_tiles = {}

        def emit_scores(idx):
            ch, qt, sk = items[idx]
            cs = slice(ch * SQC, (ch + 1) * SQC)
            sc = scps.tile([128, 2, SQC], f32, name="sc", tag="sc")
            for hh in range(2):
                hp = 64 * hh
                nc.tensor.matmul(
                    sc[:, hh, :],
                    kT_sb[hp:hp + 64, sk * 128:(sk + 1) * 128],
                    qT_sb[hp:hp + 64, qt, cs],
                    start=True, stop=True)
            sc_tiles[idx] = sc

        def evacuate_av(av):
            den, orw = [None, None], [None, None]
            for hh in range(2):
                den[hh] = p2ev.tile([1, SQC], f32, name=f"den{hh}", tag=f"den{hh}")
                nc.vector.tensor_copy(out=den[hh], in_=av[hh][HD:HD + 1, :])
                orw[hh] = p2ev.tile([HD, SQC], bf16, name=f"orw{hh}", tag=f"orw{hh}")
                with nc.allow_low_precision(reason="bf16 attn out"):
                    nc.vector.tensor_copy(out=orw[hh], in_=av[hh][0:HD, :])
            return den, orw

        def make_epilogue(ch, qt, den, orw):
            cs = slice(ch * SQC, (ch + 1) * SQC)

            def epi():
                for hh in range(2):
                    rf = p2ev.tile([1, SQC], f32, name=f"rf{hh}", tag=f"rf{hh}")
                    with nc.allow_low_precision(reason="softmax recip ~51ulp"):
                        nc.vector.reciprocal_approx_fast(out=rf, in_=den[hh])
                    rec = p2ev.tile([1, SQC], bf16, name=f"rec{hh}", tag=f"rec{hh}")
                    with nc.allow_low_precision(reason="bf16 recip bcast"):
                        nc.vector.tensor_copy(out=rec, in_=rf)
                    bc = mips.tile([128, SQC], f32, name="bc", tag="mip")
                    nc.tensor.matmul(
                        bc[0:HD, :], ones_bf, rec, start=True, stop=True)
                    bc_sb = p2ev.tile([HD, SQC], bf16, name=f"bcs{hh}", tag=f"bcs{hh}")
                    with nc.allow_low_precision(reason="bf16 recip bcast"):
                        nc.vector.tensor_copy(out=bc_sb, in_=bc[0:HD, :])
                    with nc.allow_low_precision(reason="bf16 attn out"):
                        nc.vector.tensor_mul(
                            out=oT_sb[64 * hh:64 * hh + 64, qt, cs],
                            in0=orw[hh], in1=bc_sb)
            return epi

        pending_epi = None
        av = None
        for idx, (ch, qt, sk) in enumerate(items):
            item = 2 * ch + qt
            for fn in pre.get((item, sk), ()):
                fn()
            if sk == 0:
                if idx == 0:
                    emit_scores(0)
                    emit_scores(1)
                av = [avps.tile([HD + 1, SQC], f32, name=f"av{hh}", tag="av")
                      for hh in range(2)]
            e_sb = eps.tile([128, 2, SQC], bf16, name="e_sb")
            nc.scalar.activation(
                out=e_sb, in_=sc_tiles.pop(idx),
                func=EXP, bias=bias_exp, scale=1.0)
            # eligible work first; the exp(i)-gated score lookahead goes
            # LAST so it doesn't head-of-line-block the in-order PE FIFO
            for hh in range(2):
                bi = nc.tensor.matmul(
                    av[hh][:, :], v1_sb[:, sk, :], e_sb[:, hh, :],
                    start=(sk == 0), stop=(sk == T - 1))
                if hh:
                    _noldw(bi)
            if sk == 1 and pending_epi is not None:
                pending_epi()
                pending_epi = None
            # one filler unit covers the exp(idx) latency, then the score
            # lookahead (gated on exp(idx) via the sc-slot WAR), then the
            # rest — keeps both the PE fed and the ACT stream primed
            slot_fillers = post.get((item, sk), ())
            for fn in slot_fillers[:1]:
                fn()
            if idx + 2 < len(items):
                emit_scores(idx + 2)
            for fn in slot_fillers[1:]:
                fn()
            if sk == T - 1:
                den, orw = evacuate_av(av)
                pending_epi = make_epilogue(ch, qt, den, orw)
        pending_epi()
        # tail: remaining qt0 halves + all qt1 halves of the last chunk
        tail_tasks = [(t, oc, 0) for t, oc in th[14:]] + \
                     [(t, oc, 1) for t, oc in th]
        for i, (t, oc, qt) in enumerate(tail_tasks):
            oproj_half(t, oc, qt, tail_idx=i)

    nc.compile()
    return nc


def _get_nc():
    if "nc" not in _compiled:
        _compiled["nc"] = build_gqa()
    return _compiled["nc"]


def _pm(a):
    """[KO*128, M] -> partition-major [128, KO*M] (row p holds all ko chunks)."""
    ko = a.shape[0] // 128
    return np.ascontiguousarray(
        a.reshape(ko, 128, a.shape[1]).transpose(1, 0, 2).reshape(128, -1))


def _shard_inputs(x, w_q, w_k, w_v, w_o):
    import ml_dtypes

    bf = ml_dtypes.bfloat16
    x = np.asarray(x, dtype=np.float32)
    w_q = np.asarray(w_q, dtype=np.float32)
    w_k = np.asarray(w_k, dtype=np.float32)
    w_v = np.asarray(w_v, dtype=np.float32)
    w_o = np.asarray(w_o, dtype=np.float32)
    xT = np.ascontiguousarray(x.reshape(S, D).T)
    xTp = _pm(xT).astype(bf)
    in_maps = []
    for c in range(N_CORES):
        wkv = np.concatenate(
            [w_k[:, c * HD:(c + 1) * HD], w_v[:, c * HD:(c + 1) * HD]], axis=1)
        wq = w_q[:, c * QDIM:(c + 1) * QDIM] * np.float32(SCALE)
        in_maps.append({
            "xTp": xTp,
            "wqp0": _pm(wq[:, 0:128]).astype(bf),
            "wqp1": _pm(wq[:, 128:256]).astype(bf),
            "wkvp": _pm(wkv).astype(bf),
            "wop": _pm(w_o[c * QDIM:(c + 1) * QDIM, :]).astype(bf),
        })
    return in_maps


def kernel(x, w_q, w_k, w_v, w_o):
    from concourse.bass_utils import run_bass_kernel_spmd

    nc = _get_nc()
    in_maps = _shard_inputs(x, w_q, w_k, w_v, w_o)
    res = run_bass_kernel_spmd(nc, in_maps, list(range(N_CORES)))
    acc = np.zeros((S, D), dtype=np.float64)
    for r in res.results:
        acc += r["out"].astype(np.float64)
        acc[S - SQC:] += r["out2"].astype(np.float64)
    return acc.astype(np.float32).reshape(1, S, D)


# revision 24
# speedup vs baseline: 1.0023x; 1.0023x over previous
"""GroupedQueryAttention kernel for 8 Trainium2 NeuronCores.

Sharding: tensor-parallel over KV groups (core c owns group c = 4 query
heads x 64): column shards of w_q/w_k/w_v, row shard of w_o; x
replicated (bf16, pre-transposed, partition-major); each core writes a
partial bf16 output that the host sums.

v2 design (vs the 290us flat-pipeline baseline):
- The ACT (Scalar) engine's softmax exp stream is the long pole:
  128 ACTIVATEs x ~1.15us = ~147us that cannot be reduced (exp is
  ACT-only, 1 elem/cycle/lane).  So the kernel starts that stream as
  early as the DMA allows (~19us instead of ~55us) and hides ALL other
  PE work (KV/Q projections for chunks 1-3, Q(qt1), V transposes,
  o-proj) inside the exp-paced slack via a static DMA-aware filler
  schedule.
- x is DMA'd seq-chunk-major so the chunk-0 KV+Q0 projections stream
  against the DMA and the first scores fire as soon as chunk 0 lands.
- A dummy exp ACTIVATE at t=0 pulls the ~2.7us ACT table load into the
  DMA-startup shadow.
- Steady state: ACT does nothing but exp; all psum evacuations (y
  tiles, kT/vT/qT casts, epilogue) run on the DVE; tail o-proj casts
  alternate DVE/ACT.
- PSUM budget (8 banks): scores 2x[128,2,512] (4) + AV 2x[65,512] (2)
  + filler/o-proj mip ring x2 (2).  The mip ring is strict
  round-robin, so filler emission follows a parity-safe order: held
  projection accumulators alternate slots and transients come in
  windows where the previous occupant is free.

Layouts per core (S=2048, D=2048, 4 heads of 64):
  xT_sb  [128, 16, 2048] bf16   x^T k-tiles (host partition-major)
  qT_sb  [128, 2, 2048]  bf16   Q^T; head h -> partitions 64*(h%2), slot h//2
  kT_sb  [128, 2048]     bf16   K^T duplicated on both partition halves
  v1_sb  [128, 16, 65]   bf16   [V | ones] natural layout per sk tile
  oT_sb  [128, 2, 2048]  bf16   normalized attention out (same map as qT)
  out    [2048, 2048]    bf16   partial output, host-summed
"""

import numpy as np

S = 2048
D = 2048
N_CORES = 8
HD = 64
HPG = 4
QDIM = HPG * HD           # 256
SCALE = 1.0 / 8.0         # 1/sqrt(HD)
SQC = 512                 # seq chunk (psum bank width in f32)
NCH = S // SQC            # 4
T = S // 128              # 16 sk tiles
KO = D // 128             # 16 contraction tiles
QT = QDIM // 128          # 2 q partition tiles (= head pairs)

_compiled = {}


def _noldw(bi):
    bi.ins.ldweights = False
    return bi


def build_gqa(debug=False):
    import concourse.tile as tile
    from concourse import bacc, mybir
    from concourse.masks import make_identity
    from contextlib import ExitStack

    f32 = mybir.dt.float32
    bf16 = mybir.dt.bfloat16
    EXP = mybir.ActivationFunctionType.Exp

    nc = bacc.Bacc(None, target_bir_lowering=False, debug=debug)
    xTp = nc.declare_dram_parameter("xTp", [128, KO * S], bf16, isOutput=False)
    wqp0 = nc.declare_dram_parameter("wqp0", [128, KO * 128], bf16, isOutput=False)
    wqp1 = nc.declare_dram_parameter("wqp1", [128, KO * 128], bf16, isOutput=False)
    wkvp = nc.declare_dram_parameter("wkvp", [128, KO * 2 * HD], bf16, isOutput=False)
    wop = nc.declare_dram_parameter("wop", [128, QT * D], bf16, isOutput=False)
    out = nc.declare_dram_parameter("out", [S, D], bf16, isOutput=True)
    # qt0-only partial of the last chunk's o-proj (host adds it to out's
    # qt1-only last-chunk rows) — lets item7 hoist half the o-proj tail
    out2 = nc.declare_dram_parameter("out2", [SQC, D], bf16, isOutput=True)

    with tile.TileContext(nc) as tc, ExitStack() as ctx:
        const = ctx.enter_context(tc.tile_pool(name="const", bufs=1))
        persist = ctx.enter_context(tc.tile_pool(name="persist", bufs=1))

        ident = const.tile([128, 128], bf16)
        ones_bf = const.tile([1, HD], bf16)
        bias_exp = const.tile([128, 1], f32)
        warm = const.tile([128, 1], f32)
        # dummy exp ASAP: forces the ACT table load at t~0, under the DMA
        # startup shadow (otherwise it serializes before the first real exp)
        nc.vector.memset(bias_exp, -8.0)
        nc.scalar.activation(out=warm, in_=bias_exp, func=EXP, bias=0.0, scale=1.0)
        make_identity(nc, ident)
        nc.vector.memset(ones_bf, 1.0)

        xT_sb = persist.tile([128, KO, S], bf16)
        qT_sb = persist.tile([128, QT, S], bf16)
        kT_sb = persist.tile([128, S], bf16)
        v1_sb = persist.tile([128, T, HD + 1], bf16)
        oT_sb = persist.tile([128, QT, S], bf16)
        wkv_sb = persist.tile([128, KO, 2 * HD], bf16)
        wq_sb = persist.tile([128, QT, KO, 128], bf16)
        wo_sb = persist.tile([128, QT, D], bf16)
        vT_tmp = persist.tile([64, S], bf16)

        nc.vector.memset(v1_sb[:, :, HD:HD + 1], 1.0)

        # ---------------- input DMAs ------------------------------------
        # seq-chunk-major so compute can stream against DMA arrival.
        # Few, large descriptors (sync-engine issue is ~0.6us each).
        xTr = xTp[:].rearrange("p (ko s) -> p ko s", ko=KO)

        def dma_x(ch, ko_lo, ko_hi):
            cs = slice(ch * SQC, (ch + 1) * SQC)
            nc.sync.dma_start(
                out=xT_sb[:, ko_lo:ko_hi, cs], in_=xTr[:, ko_lo:ko_hi, cs])

        nc.sync.dma_start(out=wkv_sb, in_=wkvp[:].rearrange("p (ko m) -> p ko m", ko=KO))
        nc.sync.dma_start(
            out=wq_sb[:, 0], in_=wqp0[:].rearrange("p (ko m) -> p ko m", ko=KO))
        for g in range(4):                      # chunk 0 in 4-ko sub-DMAs
            dma_x(0, 4 * g, 4 * g + 4)
        dma_x(1, 0, 8)
        dma_x(1, 8, KO)
        nc.sync.dma_start(
            out=wq_sb[:, 1], in_=wqp1[:].rearrange("p (ko m) -> p ko m", ko=KO))
        dma_x(2, 0, 8)
        dma_x(2, 8, KO)
        dma_x(3, 0, 8)
        dma_x(3, 8, KO)
        nc.sync.dma_start(out=wo_sb, in_=wop[:].rearrange("p (qt m) -> p qt m", qt=QT))

        # ---------------- pools -----------------------------------------
        scps = ctx.enter_context(tc.tile_pool(name="scps", bufs=2, space="PSUM"))
        avps = ctx.enter_context(tc.tile_pool(name="avps", bufs=2, space="PSUM"))
        mips = ctx.enter_context(tc.tile_pool(name="mips", bufs=2, space="PSUM"))
        eps = ctx.enter_context(tc.tile_pool(name="eps", bufs=8))
        p2ev = ctx.enter_context(tc.tile_pool(name="p2ev", bufs=4))
        ypool = ctx.enter_context(tc.tile_pool(name="ypool", bufs=4))

        # ---------------- filler building blocks ------------------------
        proj_state = {}

        def kv_slice(ch, kos):
            cs = slice(ch * SQC, (ch + 1) * SQC)
            if kos[0] == 0:
                proj_state[("kv", ch)] = mips.tile(
                    [128, SQC], f32, name=f"kv{ch}", tag="mip")
            ps = proj_state[("kv", ch)]
            for ko in kos:
                nc.tensor.matmul(
                    ps, wkv_sb[:, ko, :], xT_sb[:, ko, cs],
                    start=(ko == 0), stop=(ko == KO - 1))

        def kv_cast(ch):
            cs = slice(ch * SQC, (ch + 1) * SQC)
            ps = proj_state.pop(("kv", ch))
            nc.vector.tensor_copy(out=kT_sb[0:64, cs], in_=ps[0:64, :])
            nc.vector.tensor_copy(out=kT_sb[64:128, cs], in_=ps[0:64, :])
            nc.vector.tensor_copy(out=vT_tmp[:, cs], in_=ps[64:128, :])

        def q_slice(qt, ch, kos):
            cs = slice(ch * SQC, (ch + 1) * SQC)
            if kos[0] == 0:
                proj_state[("q", qt, ch)] = mips.tile(
                    [128, SQC], f32, name=f"q{qt}{ch}", tag="mip")
            ps = proj_state[("q", qt, ch)]
            for ko in kos:
                nc.tensor.matmul(
                    ps, wq_sb[:, qt, ko, :], xT_sb[:, ko, cs],
                    start=(ko == 0), stop=(ko == KO - 1))

        def q_cast(qt, ch):
            cs = slice(ch * SQC, (ch + 1) * SQC)
            ps = proj_state.pop(("q", qt, ch))
            nc.vector.tensor_copy(out=qT_sb[:, qt, cs], in_=ps)

        def vtrans(j):
            pt = mips.tile([128, HD], bf16, name="pt", tag="mip")
            nc.tensor.transpose(
                pt, vT_tmp[:, j * 128:(j + 1) * 128], ident[0:64, 0:64])
            nc.vector.tensor_copy(out=v1_sb[:, j, 0:HD], in_=pt)

        def oproj_half(t, oc, qt, tail_idx=-1):
            """Single-qt partial o-proj for the last chunk: qt0 goes to
            out2 (host adds), qt1 to the normal out rows."""
            ns = slice(oc * SQC, (oc + 1) * SQC)
            if tail_idx >= 0:
                pool, tag = [(mips, "mip"), (avps, "av"), (scps, "sc")][tail_idx % 3]
                py = pool.tile([128, SQC], f32, name="pyh", tag=tag)
            else:
                py = mips.tile([128, SQC], f32, name="pyh", tag="mip")
            nc.tensor.matmul(
                py, oT_sb[:, qt, t * 128:(t + 1) * 128], wo_sb[:, qt, ns],
                start=True, stop=True)
            y_sb = ypool.tile([128, SQC], bf16, name="y_sb")
            with nc.allow_low_precision(reason="bf16 partial output"):
                if tail_idx >= 0 and tail_idx % 2 == 1:
                    nc.scalar.copy(out=y_sb, in_=py)
                else:
                    nc.vector.tensor_copy(out=y_sb, in_=py)
            if qt == 0:
                dst = out2[:].rearrange("(t p) n -> p t n", p=128)[:, t - 12, ns]
            else:
                dst = out[:].rearrange("(t p) n -> p t n", p=128)[:, t, ns]
            nc.sync.dma_start(out=dst, in_=y_sb)

        def oproj(t, oc, tail_idx=-1):
            ns = slice(oc * SQC, (oc + 1) * SQC)
            if tail_idx >= 0:
                pool, tag = [(mips, "mip"), (avps, "av"), (scps, "sc")][tail_idx % 3]
                py = pool.tile([128, SQC], f32, name="py", tag=tag)
            else:
                py = mips.tile([128, SQC], f32, name="py", tag="mip")
            for qt in range(QT):
                nc.tensor.matmul(
                    py, oT_sb[:, qt, t * 128:(t + 1) * 128], wo_sb[:, qt, ns],
                    start=(qt == 0), stop=(qt == QT - 1))
            y_sb = ypool.tile([128, SQC], bf16, name="y_sb")
            with nc.allow_low_precision(reason="bf16 partial output"):
                if tail_idx >= 0 and tail_idx % 2 == 1:
                    nc.scalar.copy(out=y_sb, in_=py)
                else:
                    nc.vector.tensor_copy(out=y_sb, in_=py)
            nc.sync.dma_start(
                out=out[:].rearrange("(t p) n -> p t n", p=128)[:, t, ns],
                in_=y_sb)

        # ---------------- static filler schedule ------------------------
        # pre[(item, sk)] runs BEFORE the slot's score-lookahead emit (so
        # kv casts land ahead of the scores that read them — the PE FIFO
        # is in-order, a later-emitted dependency would deadlock it).
        # post[(item, sk)] runs after the slot's AV matmuls.
        pre, post = {}, {}

        def addp(d, item, sk, fn):
            d.setdefault((item, sk), []).append(fn)

        # kv chunks 1-3: front-loaded, DMA-gated (x chunk c lands ~17+5.4c us)
        addp(pre, 0, 0, lambda: kv_slice(1, range(0, 8)))
        addp(pre, 0, 1, lambda: kv_slice(1, range(8, KO)))
        addp(pre, 0, 1, lambda: kv_cast(1))
        addp(pre, 0, 4, lambda: kv_slice(2, range(0, 8)))
        addp(pre, 0, 5, lambda: kv_slice(2, range(8, KO)))
        addp(pre, 0, 6, lambda: kv_cast(2))
        addp(pre, 0, 8, lambda: kv_slice(3, range(0, 8)))
        addp(pre, 0, 9, lambda: kv_slice(3, range(8, KO)))
        addp(pre, 0, 10, lambda: kv_cast(3))
        # V transposes: v1[j] must exist before AV at (0, j)
        for j in range(4, 8):
            addp(post, 0, 1, lambda j=j: vtrans(j))
        for j in range(8, 12):
            addp(post, 0, 6, lambda j=j: vtrans(j))
        for j in range(12, 16):
            addp(post, 0, 10, lambda j=j: vtrans(j))
        # Q projections for later items (parity-safe slots, see docstring)
        addp(post, 0, 11, lambda: q_slice(1, 0, range(0, 6)))
        addp(post, 0, 12, lambda: q_slice(1, 0, range(6, 12)))
        addp(post, 0, 13, lambda: q_slice(1, 0, range(12, KO)))
        addp(post, 0, 13, lambda: q_cast(1, 0))
        addp(post, 0, 12, lambda: q_slice(0, 1, range(0, 6)))
        addp(post, 0, 13, lambda: q_slice(0, 1, range(6, 12)))
        addp(post, 0, 14, lambda: q_slice(0, 1, range(12, KO)))
        addp(post, 0, 14, lambda: q_cast(0, 1))
        addp(post, 1, 1, lambda: q_slice(1, 1, range(0, 6)))
        addp(post, 1, 2, lambda: q_slice(1, 1, range(6, 12)))
        addp(post, 1, 3, lambda: q_slice(1, 1, range(12, KO)))
        addp(post, 1, 3, lambda: q_cast(1, 1))
        addp(post, 1, 6, lambda: q_slice(0, 2, range(0, 6)))
        addp(post, 1, 7, lambda: q_slice(0, 2, range(6, 12)))
        addp(post, 1, 8, lambda: q_slice(0, 2, range(12, KO)))
        addp(post, 1, 8, lambda: q_cast(0, 2))
        addp(post, 2, 1, lambda: q_slice(1, 2, range(0, 6)))
        addp(post, 2, 2, lambda: q_slice(1, 2, range(6, 12)))
        addp(post, 2, 3, lambda: q_slice(1, 2, range(12, KO)))
        addp(post, 2, 3, lambda: q_cast(1, 2))
        addp(post, 3, 1, lambda: q_slice(0, 3, range(0, 6)))
        addp(post, 3, 2, lambda: q_slice(0, 3, range(6, 12)))
        addp(post, 3, 3, lambda: q_slice(0, 3, range(12, KO)))
        addp(post, 3, 3, lambda: q_cast(0, 3))
        addp(post, 4, 1, lambda: q_slice(1, 3, range(0, 6)))
        addp(post, 4, 2, lambda: q_slice(1, 3, range(6, 12)))
        addp(post, 4, 3, lambda: q_slice(1, 3, range(12, KO)))
        addp(post, 4, 3, lambda: q_cast(1, 3))
        # o-proj: ch0 spread thin over items 2-4 (their q-slice slots are
        # light), ch1 -> item5, ch2 -> item6 (one per sk); ch3's qt0-only
        # halves run during item7 (oT-ch3-qt0 exists after item6's
        # epilogue at (7,1)), qt1 halves go to the tail.
        t0 = [(tt, oc) for tt in range(4) for oc in range(NCH)]
        slots0 = [(2, sk) for sk in (5, 7, 9, 11, 13, 15)] + \
                 [(3, sk) for sk in (5, 7, 9, 11, 13, 15)] + \
                 [(4, sk) for sk in (5, 7, 9, 11)]
        for (tt, oc), (it, sk) in zip(t0, slots0):
            addp(post, it, sk, lambda tt=tt, oc=oc: oproj(tt, oc))
        for i, (t, oc) in enumerate([(4 + tt, oc) for tt in range(4) for oc in range(NCH)]):
            addp(post, 5, i, lambda t=t, oc=oc: oproj(t, oc))
        # ch2: first slot must follow epi(item5) at (6,1) -> start at sk2
        t2 = [(8 + tt, oc) for tt in range(4) for oc in range(NCH)]
        for i, (t, oc) in enumerate(t2[:14]):
            addp(post, 6, i + 2, lambda t=t, oc=oc: oproj(t, oc))
        for i, (t, oc) in enumerate(t2[14:]):
            addp(post, 7, i, lambda t=t, oc=oc: oproj(t, oc))
        th = [(12 + tt, oc) for tt in range(4) for oc in range(NCH)]
        for i, (t, oc) in enumerate(th[:14]):
            addp(post, 7, i + 2, lambda t=t, oc=oc: oproj_half(t, oc, 0))

        # ---------------- PE warm-up -------------------------------------
        # ~44 dummy matmuls on resident SBUF spanning the ~13us DMA-dead
        # head: ramps the PE p-state (0.65 -> 2.4GHz needs ~3us of
        # continuous work) so the DMA-paced prologue runs warm (379ns/mm
        # instead of 630), and keeps the clock high into item 0.
        for w in range(44):
            wp = scps.tile([128, 2, SQC], f32, name="warm", tag="sc")
            nc.tensor.matmul(
                wp[:, 0, :], ident, oT_sb[:, 0, 0:SQC],
                start=True, stop=True)

        # ---------------- prologue: chunk-0 KV + Q0, DMA-paced ----------
        for g in range(4):
            kv_slice(0, range(4 * g, 4 * g + 4))
            q_slice(0, 0, range(4 * g, 4 * g + 4))
        kv_cast(0)
        q_cast(0, 0)
        for j in range(4):
            vtrans(j)

        # ---------------- main loop: one flat pipeline -------------------
        items = [(ch, qt, sk)
                 for ch in range(NCH) for qt in range(QT) for sk in range(T)]
        sc_tiles = {}

        def emit_scores(idx):
            ch, qt, sk = items[idx]
            cs = slice(ch * SQC, (ch + 1) * SQC)
            sc = scps.tile([128, 2, SQC], f32, name="sc", tag="sc")
            for hh in range(2):
                hp = 64 * hh
                nc.tensor.matmul(
                    sc[:, hh, :],
                    kT_sb[hp:hp + 64, sk * 128:(sk + 1) * 128],
                    qT_sb[hp:hp + 64, qt, cs],
                    start=True, stop=True)
            sc_tiles[idx] = sc

        def evacuate_av(av):
            den, orw = [None, None], [None, None]
            for hh in range(2):
                den[hh] = p2ev.tile([1, SQC], f32, name=f"den{hh}", tag=f"den{hh}")
                nc.vector.tensor_copy(out=den[hh], in_=av[hh][HD:HD + 1, :])
                orw[hh] = p2ev.tile([HD, SQC], bf16, name=f"orw{hh}", tag=f"orw{hh}")
                with nc.allow_low_precision(reason="bf16 attn out"):
                    nc.vector.tensor_copy(out=orw[hh], in_=av[hh][0:HD, :])
            return den, orw

        def make_epilogue(ch, qt, den, orw):
            cs = slice(ch * SQC, (ch + 1) * SQC)

            def epi():
                for hh in range(2):
                    rf = p2ev.tile([1, SQC], f32, name=f"rf{hh}", tag=f"rf{hh}")
                    with nc.allow_low_precision(reason="softmax recip ~51ulp"):
                        nc.vector.reciprocal_approx_fast(out=rf, in_=den[hh])
                    rec = p2ev.tile([1, SQC], bf16, name=f"rec{hh}", tag=f"rec{hh}")
                    with nc.allow_low_precision(reason="bf16 recip bcast"):
                        nc.vector.tensor_copy(out=rec, in_=rf)
                    bc = mips.tile([128, SQC], f32, name="bc", tag="mip")
                    nc.tensor.matmul(
                        bc[0:HD, :], ones_bf, rec, start=True, stop=True)
                    bc_sb = p2ev.tile([HD, SQC], bf16, name=f"bcs{hh}", tag=f"bcs{hh}")
                    with nc.allow_low_precision(reason="bf16 recip bcast"):
                        nc.vector.tensor_copy(out=bc_sb, in_=bc[0:HD, :])
                    with nc.allow_low_precision(reason="bf16 attn out"):
                        nc.vector.tensor_mul(
                            out=oT_sb[64 * hh:64 * hh + 64, qt, cs],
                            in0=orw[hh], in1=bc_sb)
            return epi

        pending_epi = None
        av = None
        for idx, (ch, qt, sk) in enumerate(items):
            item = 2 * ch + qt
            for fn in pre.get((item, sk), ()):
                fn()
            if sk == 0:
                if idx == 0:
                    emit_scores(0)
                    emit_scores(1)
                av = [avps.tile([HD + 1, SQC], f32, name=f"av{hh}", tag="av")
                      for hh in range(2)]
            e_sb = eps.tile([128, 2, SQC], bf16, name="e_sb")
            nc.scalar.activation(
                out=e_sb, in_=sc_tiles.pop(idx),
                func=EXP, bias=bias_exp, scale=1.0)
            # eligible work first; the exp(i)-gated score lookahead goes
            # LAST so it doesn't head-of-line-block the in-order PE FIFO
            for hh in range(2):
                bi = nc.tensor.matmul(
                    av[hh][:, :], v1_sb[:, sk, :], e_sb[:, hh, :],
                    start=(sk == 0), stop=(sk == T - 1))
                if hh:
                    _noldw(bi)
            if sk == 1 and pending_epi is not None:
                pending_epi()
                pending_epi = None
            # one filler unit covers the exp(idx) latency, then the score
            # lookahead (gated on exp(idx) via the sc-slot WAR), then the
            # rest — keeps both the PE fed and the ACT stream primed
            slot_fillers = post.get((item, sk), ())
            for fn in slot_fillers[:1]:
                fn()
            if idx + 2 < len(items):
                emit_scores(idx + 2)
            for fn in slot_fillers[1:]:
                fn()
            if sk == T - 1:
                den, orw = evacuate_av(av)
                pending_epi = make_epilogue(ch, qt, den, orw)
        pending_epi()
        # tail: remaining qt0 halves + all qt1 halves of the last chunk
        tail_tasks = [(t, oc, 0) for t, oc in th[14:]] + \
                     [(t, oc, 1) for t, oc in th]
        for i, (t, oc, qt) in enumerate(tail_tasks):
            oproj_half(t, oc, qt, tail_idx=i)

    nc.compile()
    return nc


def _get_nc():
    if "nc" not in _compiled:
        _compiled["nc"] = build_gqa()
    return _compiled["nc"]


def _pm(a):
    """[KO*128, M] -> partition-major [128, KO*M] (row p holds all ko chunks)."""
    ko = a.shape[0] // 128
    return np.ascontiguousarray(
        a.reshape(ko, 128, a.shape[1]).transpose(1, 0, 2).reshape(128, -1))


def _shard_inputs(x, w_q, w_k, w_v, w_o):
    import ml_dtypes

    bf = ml_dtypes.bfloat16
    x = np.asarray(x, dtype=np.float32)
    w_q = np.asarray(w_q, dtype=np.float32)
    w_k = np.asarray(w_k, dtype=np.float32)
    w_v = np.asarray(w_v, dtype=np.float32)
    w_o = np.asarray(w_o, dtype=np.float32)
    xT = np.ascontiguousarray(x.reshape(S, D).T)
    xTp = _pm(xT).astype(bf)
    in_maps = []
    for c in range(N_CORES):
        wkv = np.concatenate(
            [w_k[:, c * HD:(c + 1) * HD], w_v[:, c * HD:(c + 1) * HD]], axis=1)
        wq = w_q[:, c * QDIM:(c + 1) * QDIM] * np.float32(SCALE)
        in_maps.append({
            "xTp": xTp,
            "wqp0": _pm(wq[:, 0:128]).astype(bf),
            "wqp1": _pm(wq[:, 128:256]).astype(bf),
            "wkvp": _pm(wkv).astype(bf),
            "wop": _pm(w_o[c * QDIM:(c + 1) * QDIM, :]).astype(bf),
        })
    return in_maps


def kernel(x, w_q, w_k, w_v, w_o):
    from concourse.bass_utils import run_bass_kernel_spmd

    nc = _get_nc()
    in_maps = _shard_inputs(x, w_q, w_k, w_v, w_o)
    res = run_bass_kernel_spmd(nc, in_maps, list(range(N_CORES)))
    acc = np.zeros((S, D), dtype=np.float64)
    for r in res.results:
        acc += r["out"].astype(np.float64)
        acc[S - SQC:] += r["out2"].astype(np.float64)
    return acc.astype(np.float32).reshape(1, S, D)


# revision 25
# speedup vs baseline: 1.0115x; 1.0092x over previous
"""GroupedQueryAttention kernel for 8 Trainium2 NeuronCores.

Sharding: tensor-parallel over KV groups (core c owns group c = 4 query
heads x 64): column shards of w_q/w_k/w_v, row shard of w_o; x
replicated (bf16, pre-transposed, partition-major); each core writes a
partial bf16 output that the host sums.

v2 design (vs the 290us flat-pipeline baseline):
- The ACT (Scalar) engine's softmax exp stream is the long pole:
  128 ACTIVATEs x ~1.15us = ~147us that cannot be reduced (exp is
  ACT-only, 1 elem/cycle/lane).  So the kernel starts that stream as
  early as the DMA allows (~19us instead of ~55us) and hides ALL other
  PE work (KV/Q projections for chunks 1-3, Q(qt1), V transposes,
  o-proj) inside the exp-paced slack via a static DMA-aware filler
  schedule.
- x is DMA'd seq-chunk-major so the chunk-0 KV+Q0 projections stream
  against the DMA and the first scores fire as soon as chunk 0 lands.
- A dummy exp ACTIVATE at t=0 pulls the ~2.7us ACT table load into the
  DMA-startup shadow.
- Steady state: ACT does nothing but exp; all psum evacuations (y
  tiles, kT/vT/qT casts, epilogue) run on the DVE; tail o-proj casts
  alternate DVE/ACT.
- PSUM budget (8 banks): scores 2x[128,2,512] (4) + AV 2x[65,512] (2)
  + filler/o-proj mip ring x2 (2).  The mip ring is strict
  round-robin, so filler emission follows a parity-safe order: held
  projection accumulators alternate slots and transients come in
  windows where the previous occupant is free.

Layouts per core (S=2048, D=2048, 4 heads of 64):
  xT_sb  [128, 16, 2048] bf16   x^T k-tiles (host partition-major)
  qT_sb  [128, 2, 2048]  bf16   Q^T; head h -> partitions 64*(h%2), slot h//2
  kT_sb  [128, 2048]     bf16   K^T duplicated on both partition halves
  v1_sb  [128, 16, 65]   bf16   [V | ones] natural layout per sk tile
  oT_sb  [128, 2, 2048]  bf16   normalized attention out (same map as qT)
  out    [2048, 2048]    bf16   partial output, host-summed
"""

import numpy as np

S = 2048
D = 2048
N_CORES = 8
HD = 64
HPG = 4
QDIM = HPG * HD           # 256
SCALE = 1.0 / 8.0         # 1/sqrt(HD)
SQC = 512                 # seq chunk (psum bank width in f32)
NCH = S // SQC            # 4
T = S // 128              # 16 sk tiles
KO = D // 128             # 16 contraction tiles
QT = QDIM // 128          # 2 q partition tiles (= head pairs)

_compiled = {}


def _noldw(bi):
    bi.ins.ldweights = False
    return bi


def build_gqa(debug=False):
    import concourse.tile as tile
    from concourse import bacc, mybir
    from concourse.masks import make_identity
    from contextlib import ExitStack

    f32 = mybir.dt.float32
    bf16 = mybir.dt.bfloat16
    EXP = mybir.ActivationFunctionType.Exp

    nc = bacc.Bacc(None, target_bir_lowering=False, debug=debug)
    xTp = nc.declare_dram_parameter("xTp", [128, KO * S], bf16, isOutput=False)
    wqp0 = nc.declare_dram_parameter("wqp0", [128, KO * 128], bf16, isOutput=False)
    wqp1 = nc.declare_dram_parameter("wqp1", [128, KO * 128], bf16, isOutput=False)
    wkvp = nc.declare_dram_parameter("wkvp", [128, KO * 2 * HD], bf16, isOutput=False)
    wop = nc.declare_dram_parameter("wop", [128, QT * D], bf16, isOutput=False)
    out = nc.declare_dram_parameter("out", [S, D], bf16, isOutput=True)
    # qt0-only partial of the last chunk's o-proj (host adds it to out's
    # qt1-only last-chunk rows) — lets item7 hoist half the o-proj tail
    out2 = nc.declare_dram_parameter("out2", [SQC, D], bf16, isOutput=True)

    with tile.TileContext(nc) as tc, ExitStack() as ctx:
        const = ctx.enter_context(tc.tile_pool(name="const", bufs=1))
        persist = ctx.enter_context(tc.tile_pool(name="persist", bufs=1))

        ident = const.tile([128, 128], bf16)
        ones_bf = const.tile([1, HD], bf16)
        bias_exp = const.tile([128, 1], f32)
        warm = const.tile([128, 1], f32)
        # dummy exp ASAP: forces the ACT table load at t~0, under the DMA
        # startup shadow (otherwise it serializes before the first real exp)
        nc.vector.memset(bias_exp, -8.0)
        nc.scalar.activation(out=warm, in_=bias_exp, func=EXP, bias=0.0, scale=1.0)
        make_identity(nc, ident)
        nc.vector.memset(ones_bf, 1.0)

        xT_sb = persist.tile([128, KO, S], bf16)
        qT_sb = persist.tile([128, QT, S], bf16)
        kT_sb = persist.tile([128, S], bf16)
        v1_sb = persist.tile([128, T, HD + 1], bf16)
        oT_sb = persist.tile([128, QT, S], bf16)
        wkv_sb = persist.tile([128, KO, 2 * HD], bf16)
        wq_sb = persist.tile([128, QT, KO, 128], bf16)
        wo_sb = persist.tile([128, QT, D], bf16)
        vT_tmp = persist.tile([64, S], bf16)

        nc.vector.memset(v1_sb[:, :, HD:HD + 1], 1.0)

        # ---------------- input DMAs ------------------------------------
        # seq-chunk-major so compute can stream against DMA arrival.
        # Few, large descriptors (sync-engine issue is ~0.6us each).
        xTr = xTp[:].rearrange("p (ko s) -> p ko s", ko=KO)

        def dma_x(ch, ko_lo, ko_hi):
            cs = slice(ch * SQC, (ch + 1) * SQC)
            nc.sync.dma_start(
                out=xT_sb[:, ko_lo:ko_hi, cs], in_=xTr[:, ko_lo:ko_hi, cs])

        nc.sync.dma_start(out=wkv_sb, in_=wkvp[:].rearrange("p (ko m) -> p ko m", ko=KO))
        nc.sync.dma_start(
            out=wq_sb[:, 0], in_=wqp0[:].rearrange("p (ko m) -> p ko m", ko=KO))
        for g in range(4):                      # chunk 0 in 4-ko sub-DMAs
            dma_x(0, 4 * g, 4 * g + 4)
        dma_x(1, 0, 8)
        dma_x(1, 8, KO)
        nc.sync.dma_start(
            out=wq_sb[:, 1], in_=wqp1[:].rearrange("p (ko m) -> p ko m", ko=KO))
        dma_x(2, 0, 8)
        dma_x(2, 8, KO)
        dma_x(3, 0, 8)
        dma_x(3, 8, KO)
        nc.sync.dma_start(out=wo_sb, in_=wop[:].rearrange("p (qt m) -> p qt m", qt=QT))

        # ---------------- pools -----------------------------------------
        scps = ctx.enter_context(tc.tile_pool(name="scps", bufs=2, space="PSUM"))
        avps = ctx.enter_context(tc.tile_pool(name="avps", bufs=2, space="PSUM"))
        mips = ctx.enter_context(tc.tile_pool(name="mips", bufs=2, space="PSUM"))
        eps = ctx.enter_context(tc.tile_pool(name="eps", bufs=8))
        p2ev = ctx.enter_context(tc.tile_pool(name="p2ev", bufs=4))
        ypool = ctx.enter_context(tc.tile_pool(name="ypool", bufs=4))

        # ---------------- filler building blocks ------------------------
        proj_state = {}

        def kv_slice(ch, kos):
            cs = slice(ch * SQC, (ch + 1) * SQC)
            if kos[0] == 0:
                proj_state[("kv", ch)] = mips.tile(
                    [128, SQC], f32, name=f"kv{ch}", tag="mip")
            ps = proj_state[("kv", ch)]
            for ko in kos:
                nc.tensor.matmul(
                    ps, wkv_sb[:, ko, :], xT_sb[:, ko, cs],
                    start=(ko == 0), stop=(ko == KO - 1))

        def kv_cast(ch):
            cs = slice(ch * SQC, (ch + 1) * SQC)
            ps = proj_state.pop(("kv", ch))
            nc.vector.tensor_copy(out=kT_sb[0:64, cs], in_=ps[0:64, :])
            nc.vector.tensor_copy(out=kT_sb[64:128, cs], in_=ps[0:64, :])
            nc.vector.tensor_copy(out=vT_tmp[:, cs], in_=ps[64:128, :])

        def q_slice(qt, ch, kos):
            cs = slice(ch * SQC, (ch + 1) * SQC)
            if kos[0] == 0:
                proj_state[("q", qt, ch)] = mips.tile(
                    [128, SQC], f32, name=f"q{qt}{ch}", tag="mip")
            ps = proj_state[("q", qt, ch)]
            for ko in kos:
                nc.tensor.matmul(
                    ps, wq_sb[:, qt, ko, :], xT_sb[:, ko, cs],
                    start=(ko == 0), stop=(ko == KO - 1))

        def q_cast(qt, ch):
            cs = slice(ch * SQC, (ch + 1) * SQC)
            ps = proj_state.pop(("q", qt, ch))
            nc.vector.tensor_copy(out=qT_sb[:, qt, cs], in_=ps)

        def vtrans(j):
            pt = mips.tile([128, HD], bf16, name="pt", tag="mip")
            nc.tensor.transpose(
                pt, vT_tmp[:, j * 128:(j + 1) * 128], ident[0:64, 0:64])
            nc.vector.tensor_copy(out=v1_sb[:, j, 0:HD], in_=pt)

        def oproj_half(t, oc, qt, tail_idx=-1):
            """Single-qt partial o-proj for the last chunk: qt0 goes to
            out2 (host adds), qt1 to the normal out rows."""
            ns = slice(oc * SQC, (oc + 1) * SQC)
            if tail_idx >= 0:
                pool, tag = [(mips, "mip"), (avps, "av"), (scps, "sc")][tail_idx % 3]
                py = pool.tile([128, SQC], f32, name="pyh", tag=tag)
            else:
                py = mips.tile([128, SQC], f32, name="pyh", tag="mip")
            nc.tensor.matmul(
                py, oT_sb[:, qt, t * 128:(t + 1) * 128], wo_sb[:, qt, ns],
                start=True, stop=True)
            y_sb = ypool.tile([128, SQC], bf16, name="y_sb")
            with nc.allow_low_precision(reason="bf16 partial output"):
                if tail_idx >= 0 and tail_idx % 2 == 1:
                    nc.scalar.copy(out=y_sb, in_=py)
                else:
                    nc.vector.tensor_copy(out=y_sb, in_=py)
            if qt == 0:
                dst = out2[:].rearrange("(t p) n -> p t n", p=128)[:, t - 12, ns]
            else:
                dst = out[:].rearrange("(t p) n -> p t n", p=128)[:, t, ns]
            nc.sync.dma_start(out=dst, in_=y_sb)

        def oproj(t, oc, tail_idx=-1):
            ns = slice(oc * SQC, (oc + 1) * SQC)
            if tail_idx >= 0:
                pool, tag = [(mips, "mip"), (avps, "av"), (scps, "sc")][tail_idx % 3]
                py = pool.tile([128, SQC], f32, name="py", tag=tag)
            else:
                py = mips.tile([128, SQC], f32, name="py", tag="mip")
            for qt in range(QT):
                nc.tensor.matmul(
                    py, oT_sb[:, qt, t * 128:(t + 1) * 128], wo_sb[:, qt, ns],
                    start=(qt == 0), stop=(qt == QT - 1))
            y_sb = ypool.tile([128, SQC], bf16, name="y_sb")
            with nc.allow_low_precision(reason="bf16 partial output"):
                if tail_idx >= 0 and tail_idx % 2 == 1:
                    nc.scalar.copy(out=y_sb, in_=py)
                else:
                    nc.vector.tensor_copy(out=y_sb, in_=py)
            nc.sync.dma_start(
                out=out[:].rearrange("(t p) n -> p t n", p=128)[:, t, ns],
                in_=y_sb)

        # ---------------- static filler schedule ------------------------
        # pre[(item, sk)] runs BEFORE the slot's score-lookahead emit (so
        # kv casts land ahead of the scores that read them — the PE FIFO
        # is in-order, a later-emitted dependency would deadlock it).
        # post[(item, sk)] runs after the slot's AV matmuls.
        pre, post = {}, {}

        def addp(d, item, sk, fn):
            d.setdefault((item, sk), []).append(fn)

        # kv chunks 1-3: front-loaded, DMA-gated (x chunk c lands ~17+5.4c us)
        addp(pre, 0, 0, lambda: kv_slice(1, range(0, 8)))
        addp(pre, 0, 1, lambda: kv_slice(1, range(8, KO)))
        addp(pre, 0, 1, lambda: kv_cast(1))
        addp(pre, 0, 4, lambda: kv_slice(2, range(0, 8)))
        addp(pre, 0, 5, lambda: kv_slice(2, range(8, KO)))
        addp(pre, 0, 6, lambda: kv_cast(2))
        addp(pre, 0, 8, lambda: kv_slice(3, range(0, 8)))
        addp(pre, 0, 9, lambda: kv_slice(3, range(8, KO)))
        addp(pre, 0, 10, lambda: kv_cast(3))
        # V transposes: v1[j] must exist before AV at (0, j)
        for j in range(4, 8):
            addp(post, 0, 1, lambda j=j: vtrans(j))
        for j in range(8, 12):
            addp(post, 0, 6, lambda j=j: vtrans(j))
        for j in range(12, 16):
            addp(post, 0, 10, lambda j=j: vtrans(j))
        # Q projections for later items (parity-safe slots, see docstring)
        addp(post, 0, 11, lambda: q_slice(1, 0, range(0, 6)))
        addp(post, 0, 12, lambda: q_slice(1, 0, range(6, 12)))
        addp(post, 0, 13, lambda: q_slice(1, 0, range(12, KO)))
        addp(post, 0, 13, lambda: q_cast(1, 0))
        addp(post, 0, 12, lambda: q_slice(0, 1, range(0, 6)))
        addp(post, 0, 13, lambda: q_slice(0, 1, range(6, 12)))
        addp(post, 0, 14, lambda: q_slice(0, 1, range(12, KO)))
        addp(post, 0, 14, lambda: q_cast(0, 1))
        addp(post, 1, 1, lambda: q_slice(1, 1, range(0, 6)))
        addp(post, 1, 2, lambda: q_slice(1, 1, range(6, 12)))
        addp(post, 1, 3, lambda: q_slice(1, 1, range(12, KO)))
        addp(post, 1, 3, lambda: q_cast(1, 1))
        addp(post, 1, 6, lambda: q_slice(0, 2, range(0, 6)))
        addp(post, 1, 7, lambda: q_slice(0, 2, range(6, 12)))
        addp(post, 1, 8, lambda: q_slice(0, 2, range(12, KO)))
        addp(post, 1, 8, lambda: q_cast(0, 2))
        addp(post, 2, 1, lambda: q_slice(1, 2, range(0, 6)))
        addp(post, 2, 2, lambda: q_slice(1, 2, range(6, 12)))
        addp(post, 2, 3, lambda: q_slice(1, 2, range(12, KO)))
        addp(post, 2, 3, lambda: q_cast(1, 2))
        addp(post, 3, 1, lambda: q_slice(0, 3, range(0, 6)))
        addp(post, 3, 2, lambda: q_slice(0, 3, range(6, 12)))
        addp(post, 3, 3, lambda: q_slice(0, 3, range(12, KO)))
        addp(post, 3, 3, lambda: q_cast(0, 3))
        addp(post, 4, 1, lambda: q_slice(1, 3, range(0, 6)))
        addp(post, 4, 2, lambda: q_slice(1, 3, range(6, 12)))
        addp(post, 4, 3, lambda: q_slice(1, 3, range(12, KO)))
        addp(post, 4, 3, lambda: q_cast(1, 3))
        # o-proj: ch0 spread thin over items 2-4 (their q-slice slots are
        # light), ch1 -> item5, ch2 -> item6 (one per sk); ch3's qt0-only
        # halves run during item7 (oT-ch3-qt0 exists after item6's
        # epilogue at (7,1)), qt1 halves go to the tail.
        t0 = [(tt, oc) for tt in range(4) for oc in range(NCH)]
        slots0 = [(2, sk) for sk in (5, 7, 9, 11, 13, 15)] + \
                 [(3, sk) for sk in (5, 7, 9, 11, 13, 15)] + \
                 [(4, sk) for sk in (5, 7, 9, 11)]
        for (tt, oc), (it, sk) in zip(t0, slots0):
            addp(post, it, sk, lambda tt=tt, oc=oc: oproj(tt, oc))
        for i, (t, oc) in enumerate([(4 + tt, oc) for tt in range(4) for oc in range(NCH)]):
            addp(post, 5, i, lambda t=t, oc=oc: oproj(t, oc))
        # ch2: first slot must follow epi(item5) at (6,1) -> start at sk2
        t2 = [(8 + tt, oc) for tt in range(4) for oc in range(NCH)]
        for i, (t, oc) in enumerate(t2[:14]):
            addp(post, 6, i + 2, lambda t=t, oc=oc: oproj(t, oc))
        for i, (t, oc) in enumerate(t2[14:]):
            addp(post, 7, i, lambda t=t, oc=oc: oproj(t, oc))
        th = [(12 + tt, oc) for tt in range(4) for oc in range(NCH)]
        for i, (t, oc) in enumerate(th[:14]):
            addp(post, 7, i + 2, lambda t=t, oc=oc: oproj_half(t, oc, 0))

        # ---------------- PE warm-up -------------------------------------
        # ~44 dummy matmuls on resident SBUF spanning the ~13us DMA-dead
        # head: ramps the PE p-state (0.65 -> 2.4GHz needs ~3us of
        # continuous work) so the DMA-paced prologue runs warm (379ns/mm
        # instead of 630), and keeps the clock high into item 0.
        for w in range(16):
            wp = scps.tile([128, 2, SQC], f32, name="warm", tag="sc")
            nc.tensor.matmul(
                wp[:, 0, :], ident, oT_sb[:, 0, 0:SQC],
                start=True, stop=True)

        # ---------------- prologue: chunk-0 KV + Q0, DMA-paced ----------
        for g in range(4):
            kv_slice(0, range(4 * g, 4 * g + 4))
            q_slice(0, 0, range(4 * g, 4 * g + 4))
        kv_cast(0)
        q_cast(0, 0)
        for j in range(4):
            vtrans(j)

        # ---------------- main loop: one flat pipeline -------------------
        items = [(ch, qt, sk)
                 for ch in range(NCH) for qt in range(QT) for sk in range(T)]
        sc_tiles = {}

        def emit_scores(idx):
            ch, qt, sk = items[idx]
            cs = slice(ch * SQC, (ch + 1) * SQC)
            sc = scps.tile([128, 2, SQC], f32, name="sc", tag="sc")
            for hh in range(2):
                hp = 64 * hh
                nc.tensor.matmul(
                    sc[:, hh, :],
                    kT_sb[hp:hp + 64, sk * 128:(sk + 1) * 128],
                    qT_sb[hp:hp + 64, qt, cs],
                    start=True, stop=True)
            sc_tiles[idx] = sc

        def evacuate_av(av):
            den, orw = [None, None], [None, None]
            for hh in range(2):
                den[hh] = p2ev.tile([1, SQC], f32, name=f"den{hh}", tag=f"den{hh}")
                nc.vector.tensor_copy(out=den[hh], in_=av[hh][HD:HD + 1, :])
                orw[hh] = p2ev.tile([HD, SQC], bf16, name=f"orw{hh}", tag=f"orw{hh}")
                with nc.allow_low_precision(reason="bf16 attn out"):
                    nc.vector.tensor_copy(out=orw[hh], in_=av[hh][0:HD, :])
            return den, orw

        def make_epilogue(ch, qt, den, orw):
            cs = slice(ch * SQC, (ch + 1) * SQC)

            def epi():
                for hh in range(2):
                    rf = p2ev.tile([1, SQC], f32, name=f"rf{hh}", tag=f"rf{hh}")
                    with nc.allow_low_precision(reason="softmax recip ~51ulp"):
                        nc.vector.reciprocal_approx_fast(out=rf, in_=den[hh])
                    rec = p2ev.tile([1, SQC], bf16, name=f"rec{hh}", tag=f"rec{hh}")
                    with nc.allow_low_precision(reason="bf16 recip bcast"):
                        nc.vector.tensor_copy(out=rec, in_=rf)
                    bc = mips.tile([128, SQC], f32, name="bc", tag="mip")
                    nc.tensor.matmul(
                        bc[0:HD, :], ones_bf, rec, start=True, stop=True)
                    bc_sb = p2ev.tile([HD, SQC], bf16, name=f"bcs{hh}", tag=f"bcs{hh}")
                    with nc.allow_low_precision(reason="bf16 recip bcast"):
                        nc.vector.tensor_copy(out=bc_sb, in_=bc[0:HD, :])
                    with nc.allow_low_precision(reason="bf16 attn out"):
                        nc.vector.tensor_mul(
                            out=oT_sb[64 * hh:64 * hh + 64, qt, cs],
                            in0=orw[hh], in1=bc_sb)
            return epi

        pending_epi = None
        av = None
        for idx, (ch, qt, sk) in enumerate(items):
            item = 2 * ch + qt
            for fn in pre.get((item, sk), ()):
                fn()
            if sk == 0:
                if idx == 0:
                    emit_scores(0)
                    emit_scores(1)
                av = [avps.tile([HD + 1, SQC], f32, name=f"av{hh}", tag="av")
                      for hh in range(2)]
            e_sb = eps.tile([128, 2, SQC], bf16, name="e_sb")
            nc.scalar.activation(
                out=e_sb, in_=sc_tiles.pop(idx),
                func=EXP, bias=bias_exp, scale=1.0)
            # eligible work first; the exp(i)-gated score lookahead goes
            # LAST so it doesn't head-of-line-block the in-order PE FIFO
            for hh in range(2):
                bi = nc.tensor.matmul(
                    av[hh][:, :], v1_sb[:, sk, :], e_sb[:, hh, :],
                    start=(sk == 0), stop=(sk == T - 1))
                if hh:
                    _noldw(bi)
            if sk == 1 and pending_epi is not None:
                pending_epi()
                pending_epi = None
            # one filler unit covers the exp(idx) latency, then the score
            # lookahead (gated on exp(idx) via the sc-slot WAR), then the
            # rest — keeps both the PE fed and the ACT stream primed
            slot_fillers = post.get((item, sk), ())
            for fn in slot_fillers[:1]:
                fn()
            if idx + 2 < len(items):
                emit_scores(idx + 2)
            for fn in slot_fillers[1:]:
                fn()
            if sk == T - 1:
                den, orw = evacuate_av(av)
                pending_epi = make_epilogue(ch, qt, den, orw)
        pending_epi()
        # tail: remaining qt0 halves + all qt1 halves of the last chunk
        tail_tasks = [(t, oc, 0) for t, oc in th[14:]] + \
                     [(t, oc, 1) for t, oc in th]
        for i, (t, oc, qt) in enumerate(tail_tasks):
            oproj_half(t, oc, qt, tail_idx=i)

    nc.compile()
    return nc


def _get_nc():
    if "nc" not in _compiled:
        _compiled["nc"] = build_gqa()
    return _compiled["nc"]


def _pm(a):
    """[KO*128, M] -> partition-major [128, KO*M] (row p holds all ko chunks)."""
    ko = a.shape[0] // 128
    return np.ascontiguousarray(
        a.reshape(ko, 128, a.shape[1]).transpose(1, 0, 2).reshape(128, -1))


def _shard_inputs(x, w_q, w_k, w_v, w_o):
    import ml_dtypes

    bf = ml_dtypes.bfloat16
    x = np.asarray(x, dtype=np.float32)
    w_q = np.asarray(w_q, dtype=np.float32)
    w_k = np.asarray(w_k, dtype=np.float32)
    w_v = np.asarray(w_v, dtype=np.float32)
    w_o = np.asarray(w_o, dtype=np.float32)
    xT = np.ascontiguousarray(x.reshape(S, D).T)
    xTp = _pm(xT).astype(bf)
    in_maps = []
    for c in range(N_CORES):
        wkv = np.concatenate(
            [w_k[:, c * HD:(c + 1) * HD], w_v[:, c * HD:(c + 1) * HD]], axis=1)
        wq = w_q[:, c * QDIM:(c + 1) * QDIM] * np.float32(SCALE)
        in_maps.append({
            "xTp": xTp,
            "wqp0": _pm(wq[:, 0:128]).astype(bf),
            "wqp1": _pm(wq[:, 128:256]).astype(bf),
            "wkvp": _pm(wkv).astype(bf),
            "wop": _pm(w_o[c * QDIM:(c + 1) * QDIM, :]).astype(bf),
        })
    return in_maps


def kernel(x, w_q, w_k, w_v, w_o):
    from concourse.bass_utils import run_bass_kernel_spmd

    nc = _get_nc()
    in_maps = _shard_inputs(x, w_q, w_k, w_v, w_o)
    res = run_bass_kernel_spmd(nc, in_maps, list(range(N_CORES)))
    acc = np.zeros((S, D), dtype=np.float64)
    for r in res.results:
        acc += r["out"].astype(np.float64)
        acc[S - SQC:] += r["out2"].astype(np.float64)
    return acc.astype(np.float32).reshape(1, S, D)
